# revision 56
# baseline (speedup 1.0000x reference)
"""AxialAttention Trainium2 Bass kernel (v4).

Problem: q,k,v of shape (4, 8, 16, 32, 32, 64) = (b, heads, t, h, w, d),
attention along the h axis (axis 3), softmax over keys, out same shape.

The computation is 512 independent "slabs" (b, heads, t), each a batch of
w=32 independent length-32 attention problems with head dim 64.  64 slabs
per NeuronCore (8 cores), processed in "quads" (4 slabs = 128 partitions).

Design notes:
  - Host pre-transposes Q and K to d-major layout; every DMA is a single
    contiguous 4KB-per-partition transfer per quad.
  - Inputs stream on two hardware DMA queues (qt+v on Sync's queue, kt
    on Scalar's; the prologue's kt 4-7 ride Sync so Scalar's engine
    never blocks in semaphore-recycle waits ahead of exp).  Outputs go
    per-group (0.25MB) on GpSimd's queue.
  - Scores: per (slab j, w) one K=64 matmul; the four slabs of a quad
    are packed as two "pair" tiles [128=(jj,d64), ...] placed at PE
    quadrants (64*jj, 32*j), producing psS [128=(j,k), (w,q)] in PSUM.
  - exp on ScalarE over [128, 512] tiles (scale = 1/sqrt(64)).
  - Softmax denominator: one N=512 matmul per 16-w group with a constant
    block-diagonal ones matrix as weights.  Since every partition of a
    band carries the same value, RD = 1/psD aligns with E partition-for-
    partition, and the normalize is fused into the block-diagonal
    scatter E2bd[(j,k),(w,j,q)] = E * RD on DVE.
  - PV: ONE matmul per w with the 128x128 block-diagonal E2bd as
    weights and rhs=V natural [128=(j,k), d] -> psPV [128=(j,q), d].
  - psPV copied (cast) to bf16 out_sb on ScalarE; host casts to fp32.
  - Software pipeline: PV lags scores by 3 groups, denominators by 1;
    loads prefetch 8 quads ahead.
"""

import os
import sys
import numpy as np

for _p in ("/root/.axon_site/_ro/trn_rl_repo", "/opt/trn_rl_repo"):
    if os.path.isdir(_p) and _p not in sys.path:
        sys.path.append(_p)

B, NH, T, H, W, D = 4, 8, 16, 32, 32, 64
N_CORES = 8
NSLAB = B * NH * T  # 512
NSLAB_CORE = NSLAB // N_CORES  # 64
NQUAD = NSLAB_CORE // 4  # 16
NGRP = 2 * NQUAD  # 32 16-w score/exp/pv groups per core

_CACHED_NC = None


def _build_nc():
    import concourse.bacc as bacc
    import concourse.mybir as mybir
    from concourse import tile

    dt = mybir.dt

    nc = bacc.Bacc("TRN2", target_bir_lowering=False, debug=False,
                   num_devices=N_CORES)
    # (quad, (jj,d64), pair, (w,q))
    qt_in = nc.dram_tensor("qt_in", [NQUAD, 128, 2, W * 32], dt.bfloat16,
                           kind="ExternalInput").ap()
    kt_in = nc.dram_tensor("kt_in", [NQUAD, 128, 2, W * 32], dt.bfloat16,
                           kind="ExternalInput").ap()
    # (quad, (j,k=h), (w,d))
    v_in = nc.dram_tensor("v_in", [NQUAD, 128, W * D], dt.bfloat16,
                          kind="ExternalInput").ap()
    # (quad, (j,q=h), (w,d))
    o_out = nc.dram_tensor("o_out", [NQUAD, 128, W * D], dt.bfloat16,
                           kind="ExternalOutput").ap()

    scale = 1.0 / float(np.sqrt(D))

    with tile.TileContext(nc) as tc:
        with tc.tile_pool(name="io", bufs=8) as io_pool, \
             tc.tile_pool(name="kk", bufs=8) as k_pool, \
             tc.tile_pool(name="vv", bufs=8) as v_pool, \
             tc.tile_pool(name="ee", bufs=5) as e_pool, \
             tc.tile_pool(name="e2", bufs=6) as e2_pool, \
             tc.tile_pool(name="rr", bufs=3) as r_pool, \
             tc.tile_pool(name="oo", bufs=6) as o_pool, \
             tc.tile_pool(name="cs", bufs=1) as c_pool, \
             tc.tile_pool(name="ps_sc", bufs=3, space="PSUM") as ps_sc, \
             tc.tile_pool(name="ps_d", bufs=2, space="PSUM") as ps_d, \
             tc.tile_pool(name="ps_pv", bufs=3, space="PSUM") as ps_pv:

            # Constant block-diagonal ones [128=(j,k), 128=(j,r)] used to
            # compute softmax denominators via one matmul per group.
            ones_bd = c_pool.tile([128, 128], dt.bfloat16, name="ones_bd")
            nc.gpsimd.memset(ones_bd[:, :], 0.0)
            for j in range(4):
                nc.gpsimd.memset(
                    ones_bd[32 * j:32 * j + 32, 32 * j:32 * j + 32], 1.0)

            qstate = {}
            gstate = {}

            def emit_loads(g, kt_eng=None, split=False, defer_v=False):
                QT = io_pool.tile([128, 2, W * 32], dt.bfloat16, name="QT")
                KT = k_pool.tile([128, 2, W * 32], dt.bfloat16, name="KT")
                V4 = v_pool.tile([128, W * D], dt.bfloat16, name="V4")
                if split:
                    # Pair-granular transfers (pair 0 further split in
                    # w-halves) so the first scores matmuls can start as
                    # soon as the first 128KB lands.
                    hw = W * 32 // 2
                    for lo, hi in ((0, hw), (hw, 2 * hw)):
                        nc.scalar.dma_start(out=KT[:, 0, lo:hi],
                                            in_=kt_in[g, :, 0, lo:hi])
                        nc.sync.dma_start(out=QT[:, 0, lo:hi],
                                          in_=qt_in[g, :, 0, lo:hi])
                    nc.scalar.dma_start(out=KT[:, 1, :],
                                        in_=kt_in[g, :, 1])
                    nc.sync.dma_start(out=QT[:, 1, :],
                                      in_=qt_in[g, :, 1])
                else:
                    (kt_eng or nc.scalar).dma_start(out=KT[:, :, :],
                                                    in_=kt_in[g])
                    nc.sync.dma_start(out=QT[:, :, :], in_=qt_in[g])
                if not defer_v:
                    nc.sync.dma_start(out=V4[:, :], in_=v_in[g])
                out_sb = o_pool.tile([128, W * D], dt.bfloat16,
                                     name="out_sb")
                qstate[g] = dict(QT=QT, KT=KT, V4=V4, out_sb=out_sb)

            def emit_v(g):
                nc.sync.dma_start(out=qstate[g]["V4"][:, :], in_=v_in[g])

            def _score_mm(psS, QT, KT, wl, w, j):
                p, jj = divmod(j, 2)
                nc.tensor.matmul(
                    psS[32 * j:32 * j + 32, wl, :],
                    lhsT=KT[64 * jj:64 * jj + 64, p, 32 * w:32 * w + 32],
                    rhs=QT[64 * jj:64 * jj + 64, p, 32 * w:32 * w + 32],
                    start=True, stop=True,
                    tile_position=(64 * jj, 32 * j))

            def emit_scores(i, pv_i=None, pair_major=False,
                            split_exp=False, denom_i=None):
                g, grp = divmod(i, 2)
                qs = qstate[g]
                QT, KT = qs["QT"], qs["KT"]
                pv = _pv_parts(pv_i) if pv_i is not None else None
                psS = ps_sc.tile([128, 16, 32], dt.float32, name="psS")
                E = e_pool.tile([128, 16, 32], dt.bfloat16, name="E")
                if pair_major:
                    # quad 0 startup: emit all pair-0 matmuls first so
                    # compute starts as soon as the first half of QT/KT
                    # lands.
                    for j in range(4):
                        for wl in range(16):
                            _score_mm(psS, QT, KT, wl, 16 * grp + wl, j)
                else:
                    for wl in range(16):
                        w = 16 * grp + wl
                        for j in range(4):
                            _score_mm(psS, QT, KT, wl, w, j)
                        if pv is not None:
                            pv(wl)
                        if denom_i is not None and wl == 8:
                            # Interleave the previous group's denominator
                            # matmul mid-scores so its recip/normalize
                            # chain starts ~1.5us earlier.
                            emit_denom(denom_i)
                        if split_exp and wl == 7:
                            nc.scalar.activation(
                                E[:, 0:8, :], psS[:, 0:8, :],
                                mybir.ActivationFunctionType.Exp,
                                scale=scale)
                if split_exp:
                    nc.scalar.activation(
                        E[:, 8:16, :], psS[:, 8:16, :],
                        mybir.ActivationFunctionType.Exp, scale=scale)
                else:
                    nc.scalar.activation(
                        E[:, :, :], psS[:, :, :],
                        mybir.ActivationFunctionType.Exp, scale=scale)
                gstate[i] = dict(E=E)

            e2bd_allocs = [0]

            def emit_denom(i, h=None):
                gs = gstate[i]
                E = gs["E"]
                if h is None or h == 0:
                    gs["psD"] = ps_d.tile([128, 16, 32], dt.float32,
                                          name="psD")
                    gs["RD"] = r_pool.tile([128, 16, 32], dt.float32,
                                           name="RD")
                    gs["RDb"] = r_pool.tile([128, 16, 32], dt.bfloat16,
                                            name="RDb")
                    E2bd = e2_pool.tile([128, 16, 4, 32], dt.bfloat16,
                                        name="E2bd")
                    if e2bd_allocs[0] < 6:
                        e2bd_allocs[0] += 1
                        nc.gpsimd.memset(E2bd[:, :, :, :], 0.0)
                    gs["E2bd"] = E2bd
                psD, RD, RDb, E2bd = (gs["psD"], gs["RD"], gs["RDb"],
                                      gs["E2bd"])
                sl = slice(None) if h is None else slice(8 * h, 8 * h + 8)
                nc.tensor.matmul(
                    psD[:, sl, :], lhsT=ones_bd[:, :], rhs=E[:, sl, :],
                    start=True, stop=True)
                nc.vector.reciprocal_approx_fast(out=RD[:, sl, :],
                                                 in_=psD[:, sl, :])
                # Alternate the fp32->bf16 cast between Scalar and Vector
                # to balance the two engines' per-group softmax load.
                if i % 2 == 0:
                    nc.scalar.copy(RDb[:, sl, :], RD[:, sl, :])
                else:
                    nc.vector.tensor_copy(RDb[:, sl, :], RD[:, sl, :])
                # Normalized E scattered into a block-diagonal weight tile
                # [128=(j,k), (w, j', q)]; off-diagonal blocks stay zero
                # (zeroed once per pool buffer).
                for j in range(4):
                    nc.vector.tensor_mul(
                        E2bd[32 * j:32 * j + 32, sl, j, :],
                        E[32 * j:32 * j + 32, sl, :],
                        RDb[32 * j:32 * j + 32, sl, :])

            def _pv_parts(i, split_out=False):
                g, grp = divmod(i, 2)
                qs = qstate[g]
                gs = gstate.pop(i)
                V4, out_sb = qs["V4"], qs["out_sb"]
                E2bd = gs["E2bd"]
                state = {}
                hoff = (W * D // 2) * grp
                seg = W * D // (4 if split_out else 2)

                def step(wl):
                    half, wl8 = divmod(wl, 8)
                    if wl8 == 0:
                        state["psPV"] = ps_pv.tile([128, 8 * D],
                                                   dt.float32, name="psPV")
                    psPV = state["psPV"]
                    w = 16 * grp + wl
                    nc.tensor.matmul(
                        psPV[:, D * wl8:D * wl8 + D],
                        lhsT=E2bd[:, wl, :, :],
                        rhs=V4[:, D * w:D * w + D],
                        start=True, stop=True)
                    if wl8 == 7:
                        w0 = 16 * grp + 8 * half
                        nc.scalar.copy(
                            out_sb[:, D * w0:D * w0 + 8 * D], psPV[:, :])
                    if wl == 15 or (split_out and wl == 7):
                        base = hoff + (seg if (split_out and wl == 15)
                                       else 0)
                        eng = nc.gpsimd if g < 8 else nc.sync
                        eng.dma_start(out=o_out[g][:, base:base + seg],
                                      in_=out_sb[:, base:base + seg])
                        if wl == 15 and grp == 1:
                            qstate.pop(g)
                return step

            def emit_pv(i):
                pv = _pv_parts(i)
                for wl in range(16):
                    pv(wl)

            # Software pipeline with a 3-group PV lag so the PE queue
            # always has runnable matmuls while exp / recip / normalize
            # drain on the Scalar and Vector engines.
            # Prologue: 7 quads; the first three quads' V transfers are
            # deferred behind the QT/KT stream (V is first needed 3
            # groups into the pipeline).
            for g0 in range(3):
                emit_loads(g0, split=(g0 == 0), defer_v=True)
            for g0 in range(3):
                emit_v(g0)
            for g0 in range(3, 7):
                emit_loads(g0, kt_eng=nc.sync if g0 >= 4 else None)
            for i in range(NGRP):
                g, grp = divmod(i, 2)
                emit_scores(i, pv_i=(i - 3) if i >= 3 else None,
                            pair_major=(i == 0),
                            split_exp=(i == NGRP - 1),
                            denom_i=(i - 1) if i >= 1 else None)
                # Loads are emitted a full group after the previous
                # occupant's last reader retired, so neither trigger
                # engine ever blocks in a pool-release wait.
                if grp == 0 and g + 7 < NQUAD:
                    emit_loads(g + 7)
            # Tail: the last group's softmax chain runs in two 8-w halves
            # overlapped with the trailing PV groups to shorten the
            # post-compute drain.
            emit_denom(NGRP - 1, h=0)
            emit_pv(NGRP - 3)
            emit_denom(NGRP - 1, h=1)
            pv_last = _pv_parts(NGRP - 1, split_out=True)
            emit_pv(NGRP - 2)
            for wl in range(8):
                pv_last(wl)
            for wl in range(8, 16):
                pv_last(wl)
    nc.compile()
    return nc


def _get_nc():
    global _CACHED_NC
    if _CACHED_NC is None:
        _CACHED_NC = _build_nc()
    return _CACHED_NC


def kernel(q, k, v, decode_step=0, decode_idx=0, _trace=False):
    from concourse.bass_utils import run_bass_kernel_spmd

    import ml_dtypes
    bf16 = ml_dtypes.bfloat16
    q = np.asarray(q, dtype=np.float32).reshape(NSLAB, H, W, D).astype(bf16)
    k = np.asarray(k, dtype=np.float32).reshape(NSLAB, H, W, D).astype(bf16)
    v = np.asarray(v, dtype=np.float32).reshape(NSLAB, H, W, D).astype(bf16)

    # d-major transpose for Q/K: (slab, d, w, h); V stays natural.
    qt = np.ascontiguousarray(q.transpose(0, 3, 2, 1))
    kt = np.ascontiguousarray(k.transpose(0, 3, 2, 1))

    nc = _get_nc()
    in_maps = []
    for c in range(N_CORES):
        sl = slice(c * NSLAB_CORE, (c + 1) * NSLAB_CORE)
        # (64, 64, 32, 32) -> (quad, (jj,d), pair, (w,q)) -> [16, 128, 2, 1024]
        qtc = qt[sl].reshape(NQUAD, 2, 2 * D, W * 32).transpose(0, 2, 1, 3)
        ktc = kt[sl].reshape(NQUAD, 2, 2 * D, W * 32).transpose(0, 2, 1, 3)
        vc = v[sl].reshape(NQUAD, 128, W * D)
        in_maps.append({
            "qt_in": np.ascontiguousarray(qtc),
            "kt_in": np.ascontiguousarray(ktc),
            "v_in": np.ascontiguousarray(vc),
        })
    res = run_bass_kernel_spmd(nc, in_maps, core_ids=list(range(N_CORES)),
                               trace=_trace)
    outs = []
    for r in res.results:
        # [16, 128, 2048] = (quad, (j, h), (w, d)) -> (slab, h, w, d)
        o = np.asarray(r["o_out"]).reshape(NSLAB_CORE, H, W, D)
        outs.append(o)
    out = np.concatenate(outs, axis=0).astype(np.float32)
    out = out.reshape(B, NH, T, H, W, D)
    if _trace:
        return out, res
    return out


if __name__ == "__main__":
    rng = np.random.default_rng(0)
    shape = (B, NH, T, H, W, D)
    q = rng.standard_normal(shape, dtype=np.float32)
    k = rng.standard_normal(shape, dtype=np.float32)
    v = rng.standard_normal(shape, dtype=np.float32)
    out = kernel(q, k, v)
    print("kernel ran, out shape", out.shape)


# revision 58
# speedup vs baseline: 1.0283x; 1.0283x over previous
"""AxialAttention Trainium2 Bass kernel (v4).

Problem: q,k,v of shape (4, 8, 16, 32, 32, 64) = (b, heads, t, h, w, d),
attention along the h axis (axis 3), softmax over keys, out same shape.

The computation is 512 independent "slabs" (b, heads, t), each a batch of
w=32 independent length-32 attention problems with head dim 64.  64 slabs
per NeuronCore (8 cores), processed in "quads" (4 slabs = 128 partitions).

Design notes:
  - Host pre-transposes Q and K to d-major layout; every DMA is a single
    contiguous 4KB-per-partition transfer per quad.
  - Inputs stream on two hardware DMA queues (qt+v on Sync's queue, kt
    on Scalar's; the prologue's kt 4-7 ride Sync so Scalar's engine
    never blocks in semaphore-recycle waits ahead of exp).  Outputs go
    per-group (0.25MB) on GpSimd's queue.
  - Scores: per (slab j, w) one K=64 matmul; the four slabs of a quad
    are packed as two "pair" tiles [128=(jj,d64), ...] placed at PE
    quadrants (64*jj, 32*j), producing psS [128=(j,k), (w,q)] in PSUM.
  - exp on ScalarE over [128, 512] tiles (scale = 1/sqrt(64)).
  - Softmax denominator: one N=512 matmul per 16-w group with a constant
    block-diagonal ones matrix as weights.  Since every partition of a
    band carries the same value, RD = 1/psD aligns with E partition-for-
    partition, and the normalize is fused into the block-diagonal
    scatter E2bd[(j,k),(w,j,q)] = E * RD on DVE.
  - PV: ONE matmul per w with the 128x128 block-diagonal E2bd as
    weights and rhs=V natural [128=(j,k), d] -> psPV [128=(j,q), d].
  - psPV copied (cast) to bf16 out_sb on ScalarE; host casts to fp32.
  - Software pipeline: PV lags scores by 3 groups, denominators by 1;
    loads prefetch 8 quads ahead.
"""

import os
import sys
import numpy as np

for _p in ("/root/.axon_site/_ro/trn_rl_repo", "/opt/trn_rl_repo"):
    if os.path.isdir(_p) and _p not in sys.path:
        sys.path.append(_p)

B, NH, T, H, W, D = 4, 8, 16, 32, 32, 64
N_CORES = 8
NSLAB = B * NH * T  # 512
NSLAB_CORE = NSLAB // N_CORES  # 64
NQUAD = NSLAB_CORE // 4  # 16
NGRP = 2 * NQUAD  # 32 16-w score/exp/pv groups per core

_CACHED_NC = None


def _build_nc():
    import concourse.bacc as bacc
    import concourse.mybir as mybir
    from concourse import tile

    dt = mybir.dt

    nc = bacc.Bacc("TRN2", target_bir_lowering=False, debug=False,
                   num_devices=N_CORES)
    # (quad, (jj,d64), pair, (w,q))
    qt_in = nc.dram_tensor("qt_in", [NQUAD, 128, 2, W * 32], dt.bfloat16,
                           kind="ExternalInput").ap()
    kt_in = nc.dram_tensor("kt_in", [NQUAD, 128, 2, W * 32], dt.bfloat16,
                           kind="ExternalInput").ap()
    # (quad, (j,k=h), (w,d))
    v_in = nc.dram_tensor("v_in", [NQUAD, 128, W * D], dt.bfloat16,
                          kind="ExternalInput").ap()
    # (quad, (j,q=h), (w,d))
    o_out = nc.dram_tensor("o_out", [NQUAD, 128, W * D], dt.bfloat16,
                           kind="ExternalOutput").ap()

    scale = 1.0 / float(np.sqrt(D))

    with tile.TileContext(nc) as tc:
        with tc.tile_pool(name="io", bufs=8) as io_pool, \
             tc.tile_pool(name="kk", bufs=8) as k_pool, \
             tc.tile_pool(name="vv", bufs=8) as v_pool, \
             tc.tile_pool(name="ee", bufs=5) as e_pool, \
             tc.tile_pool(name="e2", bufs=6) as e2_pool, \
             tc.tile_pool(name="rr", bufs=3) as r_pool, \
             tc.tile_pool(name="oo", bufs=6) as o_pool, \
             tc.tile_pool(name="cs", bufs=1) as c_pool, \
             tc.tile_pool(name="ps_sc", bufs=3, space="PSUM") as ps_sc, \
             tc.tile_pool(name="ps_d", bufs=2, space="PSUM") as ps_d, \
             tc.tile_pool(name="ps_pv", bufs=3, space="PSUM") as ps_pv:

            # Constant block-diagonal ones [128=(j,k), 128=(j,r)] used to
            # compute softmax denominators via one matmul per group.
            ones_bd = c_pool.tile([128, 128], dt.bfloat16, name="ones_bd")
            nc.gpsimd.memset(ones_bd[:, :], 0.0)
            for j in range(4):
                nc.gpsimd.memset(
                    ones_bd[32 * j:32 * j + 32, 32 * j:32 * j + 32], 1.0)

            qstate = {}
            gstate = {}

            def emit_loads(g, kt_eng=None, split=False, defer_v=False):
                QT = io_pool.tile([128, 2, W * 32], dt.bfloat16, name="QT")
                KT = k_pool.tile([128, 2, W * 32], dt.bfloat16, name="KT")
                V4 = v_pool.tile([128, W * D], dt.bfloat16, name="V4")
                if split:
                    # Pair-granular transfers (pair 0 further split in
                    # w-halves) so the first scores matmuls can start as
                    # soon as the first 128KB lands.
                    hw = W * 32 // 2
                    for lo, hi in ((0, hw), (hw, 2 * hw)):
                        nc.scalar.dma_start(out=KT[:, 0, lo:hi],
                                            in_=kt_in[g, :, 0, lo:hi])
                        nc.sync.dma_start(out=QT[:, 0, lo:hi],
                                          in_=qt_in[g, :, 0, lo:hi])
                    nc.scalar.dma_start(out=KT[:, 1, :],
                                        in_=kt_in[g, :, 1])
                    nc.sync.dma_start(out=QT[:, 1, :],
                                      in_=qt_in[g, :, 1])
                else:
                    (kt_eng or nc.scalar).dma_start(out=KT[:, :, :],
                                                    in_=kt_in[g])
                    nc.sync.dma_start(out=QT[:, :, :], in_=qt_in[g])
                if not defer_v:
                    nc.sync.dma_start(out=V4[:, :], in_=v_in[g])
                out_sb = o_pool.tile([128, W * D], dt.bfloat16,
                                     name="out_sb")
                qstate[g] = dict(QT=QT, KT=KT, V4=V4, out_sb=out_sb)

            def emit_v(g):
                nc.sync.dma_start(out=qstate[g]["V4"][:, :], in_=v_in[g])

            def _score_mm(psS, QT, KT, wl, w, j):
                p, jj = divmod(j, 2)
                nc.tensor.matmul(
                    psS[32 * j:32 * j + 32, wl, :],
                    lhsT=KT[64 * jj:64 * jj + 64, p, 32 * w:32 * w + 32],
                    rhs=QT[64 * jj:64 * jj + 64, p, 32 * w:32 * w + 32],
                    start=True, stop=True,
                    tile_position=(64 * jj, 32 * j))

            def emit_scores(i, pv_i=None, pair_major=False,
                            split_exp=False, denom_i=None):
                g, grp = divmod(i, 2)
                qs = qstate[g]
                QT, KT = qs["QT"], qs["KT"]
                pv = _pv_parts(pv_i) if pv_i is not None else None
                psS = ps_sc.tile([128, 16, 32], dt.float32, name="psS")
                E = e_pool.tile([128, 16, 32], dt.bfloat16, name="E")
                if pair_major:
                    # quad 0 startup: emit all pair-0 matmuls first so
                    # compute starts as soon as the first half of QT/KT
                    # lands.
                    for j in range(4):
                        for wl in range(16):
                            _score_mm(psS, QT, KT, wl, 16 * grp + wl, j)
                else:
                    for wl in range(16):
                        w = 16 * grp + wl
                        for j in range(4):
                            _score_mm(psS, QT, KT, wl, w, j)
                        if pv is not None:
                            pv(wl)
                        if split_exp and wl == 7:
                            nc.scalar.activation(
                                E[:, 0:8, :], psS[:, 0:8, :],
                                mybir.ActivationFunctionType.Exp,
                                scale=scale)
                if split_exp:
                    nc.scalar.activation(
                        E[:, 8:16, :], psS[:, 8:16, :],
                        mybir.ActivationFunctionType.Exp, scale=scale)
                else:
                    nc.scalar.activation(
                        E[:, :, :], psS[:, :, :],
                        mybir.ActivationFunctionType.Exp, scale=scale)
                gstate[i] = dict(E=E)

            e2bd_allocs = [0]

            def emit_denom(i, h=None):
                gs = gstate[i]
                E = gs["E"]
                if h is None or h == 0:
                    gs["psD"] = ps_d.tile([128, 16, 32], dt.float32,
                                          name="psD")
                    gs["RD"] = r_pool.tile([128, 16, 32], dt.float32,
                                           name="RD")
                    gs["RDb"] = r_pool.tile([128, 16, 32], dt.bfloat16,
                                            name="RDb")
                    E2bd = e2_pool.tile([128, 16, 4, 32], dt.bfloat16,
                                        name="E2bd")
                    if e2bd_allocs[0] < 6:
                        e2bd_allocs[0] += 1
                        nc.gpsimd.memset(E2bd[:, :, :, :], 0.0)
                    gs["E2bd"] = E2bd
                psD, RD, RDb, E2bd = (gs["psD"], gs["RD"], gs["RDb"],
                                      gs["E2bd"])
                sl = slice(None) if h is None else slice(8 * h, 8 * h + 8)
                nc.tensor.matmul(
                    psD[:, sl, :], lhsT=ones_bd[:, :], rhs=E[:, sl, :],
                    start=True, stop=True)
                nc.vector.reciprocal_approx_fast(out=RD[:, sl, :],
                                                 in_=psD[:, sl, :])
                # Alternate the fp32->bf16 cast between Scalar and Vector
                # to balance the two engines' per-group softmax load.
                if i % 2 == 0:
                    nc.scalar.copy(RDb[:, sl, :], RD[:, sl, :])
                else:
                    nc.vector.tensor_copy(RDb[:, sl, :], RD[:, sl, :])
                # Normalized E scattered into a block-diagonal weight tile
                # [128=(j,k), (w, j', q)]; off-diagonal blocks stay zero
                # (zeroed once per pool buffer).
                for j in range(4):
                    nc.vector.tensor_mul(
                        E2bd[32 * j:32 * j + 32, sl, j, :],
                        E[32 * j:32 * j + 32, sl, :],
                        RDb[32 * j:32 * j + 32, sl, :])

            def _pv_parts(i, split_out=False):
                g, grp = divmod(i, 2)
                qs = qstate[g]
                gs = gstate.pop(i)
                V4, out_sb = qs["V4"], qs["out_sb"]
                E2bd = gs["E2bd"]
                state = {}
                hoff = (W * D // 2) * grp
                seg = W * D // (4 if split_out else 2)

                def step(wl):
                    half, wl8 = divmod(wl, 8)
                    if wl8 == 0:
                        state["psPV"] = ps_pv.tile([128, 8 * D],
                                                   dt.float32, name="psPV")
                    psPV = state["psPV"]
                    w = 16 * grp + wl
                    nc.tensor.matmul(
                        psPV[:, D * wl8:D * wl8 + D],
                        lhsT=E2bd[:, wl, :, :],
                        rhs=V4[:, D * w:D * w + D],
                        start=True, stop=True)
                    if wl8 == 7:
                        w0 = 16 * grp + 8 * half
                        nc.scalar.copy(
                            out_sb[:, D * w0:D * w0 + 8 * D], psPV[:, :])
                    if wl == 15 or (split_out and wl == 7):
                        base = hoff + (seg if (split_out and wl == 15)
                                       else 0)
                        eng = nc.gpsimd if g < 8 else nc.sync
                        eng.dma_start(out=o_out[g][:, base:base + seg],
                                      in_=out_sb[:, base:base + seg])
                        if wl == 15 and grp == 1:
                            qstate.pop(g)
                return step

            def emit_pv(i):
                pv = _pv_parts(i)
                for wl in range(16):
                    pv(wl)

            # Software pipeline with a 3-group PV lag so the PE queue
            # always has runnable matmuls while exp / recip / normalize
            # drain on the Scalar and Vector engines.
            # Prologue: 7 quads; the first three quads' V transfers are
            # deferred behind the QT/KT stream (V is first needed 3
            # groups into the pipeline).
            for g0 in range(3):
                emit_loads(g0, split=(g0 == 0), defer_v=True)
            for g0 in range(3):
                emit_v(g0)
            for g0 in range(3, 7):
                emit_loads(g0, kt_eng=nc.sync if g0 >= 4 else None)
            for i in range(NGRP):
                g, grp = divmod(i, 2)
                emit_scores(i, pv_i=(i - 3) if i >= 3 else None,
                            pair_major=(i == 0),
                            split_exp=(i == NGRP - 1))
                # Loads are emitted a full group after the previous
                # occupant's last reader retired, so neither trigger
                # engine ever blocks in a pool-release wait.
                if grp == 0 and g + 7 < NQUAD:
                    emit_loads(g + 7)
                if i >= 1:
                    emit_denom(i - 1)
            # Tail: the last group's softmax chain runs in two 8-w halves
            # overlapped with the trailing PV groups to shorten the
            # post-compute drain.
            emit_denom(NGRP - 1, h=0)
            emit_pv(NGRP - 3)
            emit_denom(NGRP - 1, h=1)
            pv_last = _pv_parts(NGRP - 1, split_out=True)
            emit_pv(NGRP - 2)
            for wl in range(8):
                pv_last(wl)
            for wl in range(8, 16):
                pv_last(wl)
    nc.compile()
    return nc


def _get_nc():
    global _CACHED_NC
    if _CACHED_NC is None:
        _CACHED_NC = _build_nc()
    return _CACHED_NC


def kernel(q, k, v, decode_step=0, decode_idx=0, _trace=False):
    from concourse.bass_utils import run_bass_kernel_spmd

    import ml_dtypes
    bf16 = ml_dtypes.bfloat16
    q = np.asarray(q, dtype=np.float32).reshape(NSLAB, H, W, D).astype(bf16)
    k = np.asarray(k, dtype=np.float32).reshape(NSLAB, H, W, D).astype(bf16)
    v = np.asarray(v, dtype=np.float32).reshape(NSLAB, H, W, D).astype(bf16)

    # d-major transpose for Q/K: (slab, d, w, h); V stays natural.
    qt = np.ascontiguousarray(q.transpose(0, 3, 2, 1))
    kt = np.ascontiguousarray(k.transpose(0, 3, 2, 1))

    nc = _get_nc()
    in_maps = []
    for c in range(N_CORES):
        sl = slice(c * NSLAB_CORE, (c + 1) * NSLAB_CORE)
        # (64, 64, 32, 32) -> (quad, (jj,d), pair, (w,q)) -> [16, 128, 2, 1024]
        qtc = qt[sl].reshape(NQUAD, 2, 2 * D, W * 32).transpose(0, 2, 1, 3)
        ktc = kt[sl].reshape(NQUAD, 2, 2 * D, W * 32).transpose(0, 2, 1, 3)
        vc = v[sl].reshape(NQUAD, 128, W * D)
        in_maps.append({
            "qt_in": np.ascontiguousarray(qtc),
            "kt_in": np.ascontiguousarray(ktc),
            "v_in": np.ascontiguousarray(vc),
        })
    res = run_bass_kernel_spmd(nc, in_maps, core_ids=list(range(N_CORES)),
                               trace=_trace)
    outs = []
    for r in res.results:
        # [16, 128, 2048] = (quad, (j, h), (w, d)) -> (slab, h, w, d)
        o = np.asarray(r["o_out"]).reshape(NSLAB_CORE, H, W, D)
        outs.append(o)
    out = np.concatenate(outs, axis=0).astype(np.float32)
    out = out.reshape(B, NH, T, H, W, D)
    if _trace:
        return out, res
    return out


if __name__ == "__main__":
    rng = np.random.default_rng(0)
    shape = (B, NH, T, H, W, D)
    q = rng.standard_normal(shape, dtype=np.float32)
    k = rng.standard_normal(shape, dtype=np.float32)
    v = rng.standard_normal(shape, dtype=np.float32)
    out = kernel(q, k, v)
    print("kernel ran, out shape", out.shape)
